# revision 47
# baseline (speedup 1.0000x reference)
"""Causal self-attention with ALiBi, sharded over 8 TRN2 NeuronCores.

Sharding: core c -> batch b = c//4, head group g = c%4 (4 heads each).
Each core computes QKV projection for its heads, causal attention, and the
partial output projection (w_proj rows of its heads). Host sums the 4
partials per batch and adds b_proj.

Kernel math tricks (all folded into matmuls so softmax is one exp pass):
  - scores are computed TRANSPOSED (s on partitions, t free) so exp(S^T)=P^T
    lands exactly in the lhsT layout the P@V matmul needs.
  - ALiBi bias slope*s, the stability offset -(slope*t + c), and the /sqrt(D)
    scale are folded into 3 extra contraction rows of the QK^T matmul
    (q' = [q/8, 1, 1, -(slope*t+c)], k' = [k, hi, lo, 1] with hi+lo an exact
    split of slope*s to survive f32r rounding).
  - V is augmented with a ones column so the softmax denominator appears as
    row 64 of the (unnormalized) y^T accumulator.
  - normalization commutes with the head-dim contraction, applied via
    reciprocal + partition broadcast before the output projection.
All matmuls run in f32r (single-pass fp32, ~1e-4 rel err).
"""

import numpy as np

B, T, C, H = 2, 2048, 1024, 16
D = C // H          # 64
HL = 4              # heads per core
NCORES = 8
COFF = 8.0          # softmax stability offset

_prog_cache = {}


def _round_keep9(x):
    """RNE to 9 explicit mantissa bits (exactly representable in f32r)."""
    b = np.asarray(x, np.float32).view(np.uint32)
    half = np.uint32(1 << 13)
    mask = np.uint32(0xFFFFFFFF) << 14
    return ((b + half) & mask).view(np.float32)


def _build_program():
    import concourse.bass as bass  # noqa: F401
    import concourse.mybir as mybir
    import concourse.tile as tile
    from concourse import bacc

    f32 = mybir.dt.float32
    f32r = mybir.dt.float32r
    EXP = mybir.ActivationFunctionType.Exp
    CPY = mybir.ActivationFunctionType.Copy

    nc = bacc.Bacc("TRN2", target_bir_lowering=False, num_devices=NCORES)

    x_in = nc.declare_dram_parameter("x", [T, C], f32r, isOutput=False)
    wqk_in = nc.declare_dram_parameter("wqk", [C, 512], f32r, isOutput=False)
    wv_in = nc.declare_dram_parameter("wv", [C, 256], f32r, isOutput=False)
    wp_in = nc.declare_dram_parameter("wp", [256, C], f32r, isOutput=False)
    bqk_in = nc.declare_dram_parameter("bqk", [128, 4], f32, isOutput=False)
    bv_in = nc.declare_dram_parameter("bv", [1, 256], f32r, isOutput=False)
    # aug rows per head: [.., 29:32, :] = the 3 aug rows ([1,1,qaug] q-side,
    # [khi,klo,1] k-side); rows 0:29 are zeros (odd-head padding).
    augq_in = nc.declare_dram_parameter("augq", [HL, 32, T], f32r, isOutput=False)
    augk_in = nc.declare_dram_parameter("augk", [HL, 32, T], f32r, isOutput=False)
    out_dram = nc.declare_dram_parameter("out", [T, C], f32, isOutput=True)

    with tile.TileContext(nc) as tc:
        with (
            tc.tile_pool(name="persist", bufs=1) as pp,
            tc.tile_pool(name="consts", bufs=1) as cp,
        ):
            # ---- constants / weights ----
            from concourse.masks import make_identity

            ident = cp.tile([128, 128], f32)
            make_identity(nc, ident)
            identr = cp.tile([128, 128], f32r)
            nc.vector.tensor_copy(identr, ident)

            # prefetch the first t-super of x before anything else so the
            # transposes (first PE work) start as early as possible
            p2 = tc.alloc_tile_pool(name="ph2", bufs=2)
            p2pt = tc.alloc_tile_pool(name="ph2pt", bufs=4)
            p3 = tc.alloc_tile_pool(name="ph3", bufs=2)
            ps2a = tc.alloc_tile_pool(name="ps2a", bufs=3, space="PSUM")
            ps2b = tc.alloc_tile_pool(name="ps2b", bufs=1, space="PSUM")
            p1a = tc.alloc_tile_pool(name="ph1a", bufs=1)
            p1b = tc.alloc_tile_pool(name="ph1b", bufs=1)
            psP = tc.alloc_tile_pool(name="psP", bufs=2, space="PSUM")
            xn0 = []
            for k in range(4):
                xt_ = p1a.tile([128, C], f32r, tag=f"xnat{k}")
                xn0.append(xt_)
            for half in range(2):
                csl = slice(512 * half, 512 * (half + 1))
                for k in range(4):
                    nc.sync.dma_start(
                        out=xn0[k][:, csl], in_=x_in[128 * k:128 * (k + 1), csl]
                    )

            wqk_sb = [cp.tile([128, 512], f32r, name=f"wqk{c}", tag=f"wqk{c}") for c in range(8)]
            for c in range(8):
                nc.sync.dma_start(out=wqk_sb[c], in_=wqk_in[128 * c:128 * (c + 1), :])
            wv_sb = [cp.tile([128, 256], f32r, name=f"wv{c}", tag=f"wv{c}") for c in range(8)]
            for c in range(8):
                nc.sync.dma_start(out=wv_sb[c], in_=wv_in[128 * c:128 * (c + 1), :])
            bqk_sb = cp.tile([128, 4], f32)
            nc.sync.dma_start(out=bqk_sb, in_=bqk_in[:, :])
            bv_sb = cp.tile([1, 256], f32r)
            nc.sync.dma_start(out=bv_sb, in_=bv_in[:, :])
            ones_t = cp.tile([1, 128], f32r)
            nc.vector.memset(ones_t.bitcast(f32), 1.0)

            # ---- persistent attention operands ----
            # Q'/K' per head: [128, T]. Even local head: rows 0-63 head data,
            # rows 64-66 augs. Odd local head: rows 61-63 augs, 64-127 data.
            QP = [pp.tile([128, T], f32r, name=f"QP{h}", tag=f"QP{h}") for h in range(HL)]
            KP = [pp.tile([128, T], f32r, name=f"KP{h}", tag=f"KP{h}") for h in range(HL)]
            # V' per s-block: [128, HL, 65] (cols 0-63 = v, col 64 = ones)
            VP = [pp.tile([128, HL, 65], f32r, name=f"VP{j}", tag=f"VP{j}") for j in range(16)]
            # normalized y^T stacked per head pair: [128, T]
            PAIR = [pp.tile([128, T], f32r, name=f"PAIR{p}", tag=f"PAIR{p}") for p in range(2)]

            for h in range(HL):
                if h % 2 == 0:
                    # rows 64-66 = augs; contraction slice [0:67]
                    nc.sync.dma_start(out=QP[h][64:67, :], in_=augq_in[h, 29:32, :])
                    nc.sync.dma_start(out=KP[h][64:67, :], in_=augk_in[h, 29:32, :])
                else:
                    # contraction slice [0:128]: rows 0-60 zero, 61-63 augs,
                    # 64-127 data (zero rows cost nothing: PE time ~ N only)
                    nc.vector.memset(QP[h][0:32, :].bitcast(f32), 0.0)
                    nc.vector.memset(KP[h][0:32, :].bitcast(f32), 0.0)
                    nc.sync.dma_start(out=QP[h][32:64, :], in_=augq_in[h, :, :])
                    nc.sync.dma_start(out=KP[h][32:64, :], in_=augk_in[h, :, :])
            for j in range(16):
                nc.vector.memset(VP[j][:, :, 64:65].bitcast(f32), 1.0)

            # ===== interleaved pipeline: projections feed attention =====
            # PSUM budget (8 banks): p1 shared proj staging (2) + scores (4)
            # + y accumulators (2); after phase-1 release, fp takes p1's banks.
            psF = [None]

            if True:
                wp_sb = [p3.tile([128, C], f32r, name=f"wp{p}", tag=f"wp{p}") for p in range(2)]
                for p in range(2):
                    nc.sync.dma_start(out=wp_sb[p], in_=wp_in[128 * p:128 * (p + 1), :])

                def emit_ts(ts):
                    if ts == 0:
                        xn = xn0
                    else:
                        xn = []
                        for k in range(4):
                            t0 = 512 * ts + 128 * k
                            xt_ = p1a.tile([128, C], f32r, tag=f"xnat{k}")
                            nc.sync.dma_start(out=xt_, in_=x_in[t0:t0 + 128, :])
                            xn.append(xt_)
                    xtc = []
                    for c in range(8):
                        tp = psP.tile([128, 512], f32, tag="p1")
                        for k in range(4):
                            nc.tensor.transpose(
                                tp[:, 128 * k:128 * (k + 1)].bitcast(f32r),
                                xn[k][:, 128 * c:128 * (c + 1)],
                                identr,
                            )
                        xc = p1b.tile([128, 512], f32r, tag=f"xtc{c}")
                        nc.scalar.activation(xc, tp, CPY)
                        xtc.append(xc)
                    for m in range(4):
                        qk = psP.tile([128, 512], f32, tag="p1")
                        for c in range(8):
                            nc.tensor.matmul(
                                qk,
                                wqk_sb[c][:, 128 * m:128 * (m + 1)],
                                xtc[c],
                                start=(c == 0),
                                stop=(c == 7),
                            )
                        dest = QP if m < 2 else KP
                        h0 = 2 * (m % 2)
                        tsl = slice(512 * ts, 512 * (ts + 1))
                        nc.vector.tensor_scalar_add(
                            dest[h0][0:64, tsl], qk[0:64, :], bqk_sb[0:64, m:m + 1]
                        )
                        nc.vector.tensor_scalar_add(
                            dest[h0 + 1][64:128, tsl], qk[64:128, :], bqk_sb[64:128, m:m + 1]
                        )
                    for k in range(4):
                        jj = 4 * ts + k
                        vp = psP.tile([128, 512], f32, tag="p1")
                        for c in range(8):
                            nc.tensor.matmul(
                                vp[:, 0:256],
                                xtc[c][:, 128 * k:128 * (k + 1)],
                                wv_sb[c],
                                start=(c == 0),
                                stop=False,
                            )
                        nc.tensor.matmul(vp[:, 0:256], ones_t, bv_sb, start=False, stop=True)
                        nc.vector.tensor_copy(
                            VP[jj][:, :, 0:64],
                            vp[:, 0:256].rearrange("p (h d) -> p h d", h=HL),
                        )

                def normalize(h, i, yt):
                    """Evacuate Y psum, divide by denominator row, store to PAIR."""
                    ysb = p2.tile([65, 512], f32, tag="ysb")
                    nc.vector.tensor_copy(ysb, yt)  # frees the psum bank fast
                    den = p2.tile([1, 512], f32, tag="den")
                    nc.sync.dma_start(out=den, in_=ysb[64:65, :])
                    rr = p2.tile([1, 512], f32, tag="rr")
                    nc.vector.reciprocal_approx_fast(out=rr, in_=den)
                    rbc = p2.tile([64, 512], f32, tag="rbc")
                    nc.gpsimd.partition_broadcast(out_ap=rbc, in_ap=rr)
                    tsl = slice(512 * i, 512 * (i + 1))
                    if h % 2 == 0:
                        nc.vector.tensor_mul(PAIR[h // 2][0:64, tsl], ysb[0:64, :], rbc)
                    else:
                        stg = p2.tile([64, 512], f32r, tag="stg")
                        nc.vector.tensor_mul(stg, ysb[0:64, :], rbc)
                        nc.sync.dma_start(out=PAIR[h // 2][64:128, tsl], in_=stg)

                def project(i):
                    """Output projection for t-blocks of t-tile i (all heads done)."""
                    for tb in range(4 * i, 4 * i + 4):
                        fp = psF[0].tile([128, 1024], f32, tag="fp")
                        tsl = slice(128 * tb, 128 * (tb + 1))
                        for n in range(2):
                            nsl = slice(512 * n, 512 * (n + 1))
                            for p in range(2):
                                nc.tensor.matmul(
                                    fp[:, nsl],
                                    PAIR[p][:, tsl],
                                    wp_sb[p][:, nsl],
                                    start=(p == 0),
                                    stop=(p == 1),
                                )
                        ob = p3.tile([128, 1024], f32, tag="ob")
                        nc.vector.tensor_copy(ob, fp)
                        nc.sync.dma_start(out=out_dram[tsl, :], in_=ob)

                # Slot h holds global heads {h*4+g : g}; the flattest slope in
                # slot h is 2^(-2(h+1)), so keys further than DELTA[h] behind
                # the query contribute < e^-32 of the softmax mass -> skip.
                DELTA = [12 * 4 ** (h + 1) for h in range(HL)]

                def emit_att(th, hs, proj_after=()):
                    tbase = 1024 * th
                    ilo_half, ihi_half = 2 * th, 2 * th + 2
                    for h in hs:
                        rows = slice(0, 67) if h % 2 == 0 else slice(0, 128)
                        Y = {}
                        started = set()

                        def front(j, i, diag):
                            """Scores + exp + causal mask for item (j, i)."""
                            n0 = 128 * (j % 4) if diag else 0
                            # full-width matmul: a PSUM start=True write that
                            # begins at a nonzero offset does not clear the
                            # region under pool-buffer reuse (stale data would
                            # be accumulated), so always write [0:512]
                            S = ps2a.tile([128, 512], f32, tag="sc")
                            nc.tensor.matmul(
                                S,
                                KP[h][rows, 128 * j:128 * (j + 1)],
                                QP[h][rows, 512 * i:512 * (i + 1)],
                                start=True,
                                stop=True,
                            )
                            PT = p2pt.tile([128, 512], f32r, tag="pt")
                            nc.scalar.activation(PT[:, n0:512], S[:, n0:512], EXP)
                            if diag:
                                nc.gpsimd.affine_select(
                                    out=PT[:, n0:n0 + 128],
                                    in_=PT[:, n0:n0 + 128],
                                    compare_op=mybir.AluOpType.is_ge,
                                    fill=0.0,
                                    base=0,
                                    pattern=[[1, 128]],
                                    channel_multiplier=-1,
                                )
                            return PT

                        def back(j, i, diag, PT):
                            """P@V accumulation (+normalize/project hooks)."""
                            ya = 128 * (j % 4) if diag else 0
                            if i not in Y:
                                Y[i] = ps2b.tile(
                                    [65, 512], f32,
                                    tag=f"yb{i % 2}", name=f"Y{h}_{i}",
                                )
                            nc.tensor.matmul(
                                Y[i][:, ya:512],
                                VP[j][:, h, :],
                                PT[:, ya:512],
                                start=(i not in started),
                                stop=(j == 4 * i + 3),
                            )
                            started.add(i)
                            if j == 4 * i + 3:
                                normalize(h, i, Y.pop(i))
                                if h == hs[-1] and i in proj_after:
                                    project(i)

                        # two-deep software pipeline over (j, i) items: the
                        # next blocks' scores run on PE while the Act engine
                        # exps this block, so P@V never waits out the full
                        # exp latency
                        items = []
                        for j in range(8 * th + 8):
                            i0 = j // 4
                            for i in range(max(i0, ilo_half), ihi_half):
                                if 128 * j + 127 >= 512 * i - DELTA[h]:
                                    items.append((j, i, i == i0))
                        PIPE = 2
                        pts = {}
                        for idx, it in enumerate(items):
                            pts[idx] = front(*it)
                            if idx >= PIPE:
                                back(*items[idx - PIPE], pts.pop(idx - PIPE))
                        for idx in range(max(0, len(items) - PIPE), len(items)):
                            back(*items[idx], pts.pop(idx))

                # --- interleaved emission ---
                emit_ts(0)
                emit_ts(1)
                emit_att(0, [0, 1])
                emit_ts(2)
                emit_att(0, [2, 3])
                emit_ts(3)
                psP.release()
                p1b.release()
                p1a.release()
                psF[0] = tc.alloc_tile_pool(name="psF", bufs=1, space="PSUM")
                project(0)
                project(1)
                emit_att(1, [0, 1, 2, 3], proj_after=(2, 3))
                psF[0].release()
                ps2b.release()
                ps2a.release()
                p3.release()
                p2pt.release()
                p2.release()




    nc.finalize()
    return nc


def _get_program():
    if "nc" not in _prog_cache:
        _prog_cache["nc"] = _build_program()
    return _prog_cache["nc"]


def _prep_core_inputs(core, x, w_attn, b_attn, w_proj):
    b, g = core // 4, core % 4
    # slot i holds global head g + 4*i (slopes grouped by magnitude per slot)
    heads = [g + 4 * i for i in range(HL)]
    qc = [slice((0 * H + h) * D, (0 * H + h) * D + D) for h in heads]
    kc = [slice((1 * H + h) * D, (1 * H + h) * D + D) for h in heads]
    vc = [slice((2 * H + h) * D, (2 * H + h) * D + D) for h in heads]

    wq = np.concatenate([w_attn[:, s] for s in qc], 1) * 0.125
    wk = np.concatenate([w_attn[:, s] for s in kc], 1)
    wqk = np.concatenate([wq, wk], 1).astype(np.float32)          # [C, 512]
    wv = np.concatenate([w_attn[:, s] for s in vc], 1).astype(np.float32)
    bq = np.concatenate([b_attn[s] for s in qc]) * 0.125
    bk = np.concatenate([b_attn[s] for s in kc])
    bqk = np.concatenate([bq, bk]).astype(np.float32).reshape(4, 128).T.copy()
    bv = np.concatenate([b_attn[s] for s in vc]).astype(np.float32)[None, :]
    wp = np.concatenate([w_proj[s, :] for s in qc], 0).astype(np.float32)  # [256, C]

    slopes = 2.0 ** (-(8.0 / H) * (np.array(heads, np.float64) + 1.0))
    pos = np.arange(T, dtype=np.float64)
    kaug = slopes[:, None] * pos[None, :]                          # [HL, T]
    khi = _round_keep9(kaug)
    klo = (kaug - khi.astype(np.float64)).astype(np.float32)
    qaug = (-(kaug + COFF)).astype(np.float32)

    augq = np.zeros((HL, 32, T), np.float32)
    augq[:, 29, :] = 1.0
    augq[:, 30, :] = 1.0
    augq[:, 31, :] = qaug
    augk = np.zeros((HL, 32, T), np.float32)
    augk[:, 29, :] = khi
    augk[:, 30, :] = klo
    augk[:, 31, :] = 1.0

    return {
        "x": np.ascontiguousarray(x[b], np.float32),
        "wqk": wqk,
        "wv": wv,
        "wp": np.ascontiguousarray(wp),
        "bqk": bqk,
        "bv": bv,
        "augq": augq,
        "augk": augk,
    }


def kernel(x, w_attn, b_attn, w_proj, b_proj, _run_kwargs=None):
    from concourse.bass_utils import run_bass_kernel_spmd

    x = np.asarray(x, np.float32)
    w_attn = np.asarray(w_attn, np.float32)
    b_attn = np.asarray(b_attn, np.float32)
    w_proj = np.asarray(w_proj, np.float32)
    b_proj = np.asarray(b_proj, np.float32)

    nc = _get_program()
    in_maps = [_prep_core_inputs(c, x, w_attn, b_attn, w_proj) for c in range(NCORES)]
    res = run_bass_kernel_spmd(
        nc, in_maps, core_ids=list(range(NCORES)), **(_run_kwargs or {})
    )
    _prog_cache["last_result"] = res

    out = np.zeros((B, T, C), np.float32)
    for c in range(NCORES):
        out[c // 4] += res.results[c]["out"]
    out += b_proj[None, None, :]
    return out



# revision 48
# speedup vs baseline: 1.0213x; 1.0213x over previous
"""Causal self-attention with ALiBi, sharded over 8 TRN2 NeuronCores.

Sharding: core c -> batch b = c//4, head group g = c%4 (4 heads each).
Each core computes QKV projection for its heads, causal attention, and the
partial output projection (w_proj rows of its heads). Host sums the 4
partials per batch and adds b_proj.

Kernel math tricks (all folded into matmuls so softmax is one exp pass):
  - scores are computed TRANSPOSED (s on partitions, t free) so exp(S^T)=P^T
    lands exactly in the lhsT layout the P@V matmul needs.
  - ALiBi bias slope*s, the stability offset -(slope*t + c), and the /sqrt(D)
    scale are folded into 3 extra contraction rows of the QK^T matmul
    (q' = [q/8, 1, 1, -(slope*t+c)], k' = [k, hi, lo, 1] with hi+lo an exact
    split of slope*s to survive f32r rounding).
  - V is augmented with a ones column so the softmax denominator appears as
    row 64 of the (unnormalized) y^T accumulator.
  - normalization commutes with the head-dim contraction, applied via
    reciprocal + partition broadcast before the output projection.
All matmuls run in f32r (single-pass fp32, ~1e-4 rel err).
"""

import numpy as np

B, T, C, H = 2, 2048, 1024, 16
D = C // H          # 64
HL = 4              # heads per core
NCORES = 8
COFF = 8.0          # softmax stability offset

_prog_cache = {}


def _round_keep9(x):
    """RNE to 9 explicit mantissa bits (exactly representable in f32r)."""
    b = np.asarray(x, np.float32).view(np.uint32)
    half = np.uint32(1 << 13)
    mask = np.uint32(0xFFFFFFFF) << 14
    return ((b + half) & mask).view(np.float32)


def _build_program():
    import concourse.bass as bass  # noqa: F401
    import concourse.mybir as mybir
    import concourse.tile as tile
    from concourse import bacc

    f32 = mybir.dt.float32
    f32r = mybir.dt.float32r
    EXP = mybir.ActivationFunctionType.Exp
    CPY = mybir.ActivationFunctionType.Copy

    nc = bacc.Bacc("TRN2", target_bir_lowering=False, num_devices=NCORES)

    x_in = nc.declare_dram_parameter("x", [T, C], f32r, isOutput=False)
    wqk_in = nc.declare_dram_parameter("wqk", [C, 512], f32r, isOutput=False)
    wv_in = nc.declare_dram_parameter("wv", [C, 256], f32r, isOutput=False)
    wp_in = nc.declare_dram_parameter("wp", [256, C], f32r, isOutput=False)
    bqk_in = nc.declare_dram_parameter("bqk", [128, 4], f32, isOutput=False)
    bv_in = nc.declare_dram_parameter("bv", [1, 256], f32r, isOutput=False)
    # aug rows per head: [.., 29:32, :] = the 3 aug rows ([1,1,qaug] q-side,
    # [khi,klo,1] k-side); rows 0:29 are zeros (odd-head padding).
    augq_in = nc.declare_dram_parameter("augq", [HL, 32, T], f32r, isOutput=False)
    augk_in = nc.declare_dram_parameter("augk", [HL, 32, T], f32r, isOutput=False)
    out_dram = nc.declare_dram_parameter("out", [T, C], f32, isOutput=True)

    with tile.TileContext(nc) as tc:
        with (
            tc.tile_pool(name="persist", bufs=1) as pp,
            tc.tile_pool(name="consts", bufs=1) as cp,
        ):
            # ---- constants / weights ----
            from concourse.masks import make_identity

            ident = cp.tile([128, 128], f32)
            make_identity(nc, ident)
            identr = cp.tile([128, 128], f32r)
            nc.vector.tensor_copy(identr, ident)

            # prefetch the first t-super of x before anything else so the
            # transposes (first PE work) start as early as possible
            p2 = tc.alloc_tile_pool(name="ph2", bufs=2)
            p2pt = tc.alloc_tile_pool(name="ph2pt", bufs=3)
            p3 = tc.alloc_tile_pool(name="ph3", bufs=2)
            ps2a = tc.alloc_tile_pool(name="ps2a", bufs=3, space="PSUM")
            ps2b = tc.alloc_tile_pool(name="ps2b", bufs=1, space="PSUM")
            p1a = tc.alloc_tile_pool(name="ph1a", bufs=1)
            p1b = tc.alloc_tile_pool(name="ph1b", bufs=1)
            psP = tc.alloc_tile_pool(name="psP", bufs=2, space="PSUM")
            xn0 = []
            for k in range(4):
                xt_ = p1a.tile([128, C], f32r, tag=f"xnat{k}")
                nc.sync.dma_start(out=xt_, in_=x_in[128 * k:128 * (k + 1), :])
                xn0.append(xt_)

            wqk_sb = [cp.tile([128, 512], f32r, name=f"wqk{c}", tag=f"wqk{c}") for c in range(8)]
            for c in range(8):
                nc.sync.dma_start(out=wqk_sb[c], in_=wqk_in[128 * c:128 * (c + 1), :])
            wv_sb = [cp.tile([128, 256], f32r, name=f"wv{c}", tag=f"wv{c}") for c in range(8)]
            for c in range(8):
                nc.sync.dma_start(out=wv_sb[c], in_=wv_in[128 * c:128 * (c + 1), :])
            bqk_sb = cp.tile([128, 4], f32)
            nc.sync.dma_start(out=bqk_sb, in_=bqk_in[:, :])
            bv_sb = cp.tile([1, 256], f32r)
            nc.sync.dma_start(out=bv_sb, in_=bv_in[:, :])
            ones_t = cp.tile([1, 128], f32r)
            nc.vector.memset(ones_t.bitcast(f32), 1.0)

            # ---- persistent attention operands ----
            # Q'/K' per head: [128, T]. Even local head: rows 0-63 head data,
            # rows 64-66 augs. Odd local head: rows 61-63 augs, 64-127 data.
            QP = [pp.tile([128, T], f32r, name=f"QP{h}", tag=f"QP{h}") for h in range(HL)]
            KP = [pp.tile([128, T], f32r, name=f"KP{h}", tag=f"KP{h}") for h in range(HL)]
            # V' per s-block: [128, HL, 65] (cols 0-63 = v, col 64 = ones)
            VP = [pp.tile([128, HL, 65], f32r, name=f"VP{j}", tag=f"VP{j}") for j in range(16)]
            # normalized y^T stacked per head pair: [128, T]
            PAIR = [pp.tile([128, T], f32r, name=f"PAIR{p}", tag=f"PAIR{p}") for p in range(2)]

            for h in range(HL):
                if h % 2 == 0:
                    # rows 64-66 = augs; contraction slice [0:67]
                    nc.sync.dma_start(out=QP[h][64:67, :], in_=augq_in[h, 29:32, :])
                    nc.sync.dma_start(out=KP[h][64:67, :], in_=augk_in[h, 29:32, :])
                else:
                    # contraction slice [0:128]: rows 0-60 zero, 61-63 augs,
                    # 64-127 data (zero rows cost nothing: PE time ~ N only)
                    nc.vector.memset(QP[h][0:32, :].bitcast(f32), 0.0)
                    nc.vector.memset(KP[h][0:32, :].bitcast(f32), 0.0)
                    nc.sync.dma_start(out=QP[h][32:64, :], in_=augq_in[h, :, :])
                    nc.sync.dma_start(out=KP[h][32:64, :], in_=augk_in[h, :, :])
            for j in range(16):
                nc.vector.memset(VP[j][:, :, 64:65].bitcast(f32), 1.0)

            # ===== interleaved pipeline: projections feed attention =====
            # PSUM budget (8 banks): p1 shared proj staging (2) + scores (4)
            # + y accumulators (2); after phase-1 release, fp takes p1's banks.
            psF = [None]

            if True:
                wp_sb = [p3.tile([128, C], f32r, name=f"wp{p}", tag=f"wp{p}") for p in range(2)]
                for p in range(2):
                    nc.sync.dma_start(out=wp_sb[p], in_=wp_in[128 * p:128 * (p + 1), :])

                def emit_ts(ts):
                    if ts == 0:
                        xn = xn0
                    else:
                        xn = []
                        for k in range(4):
                            t0 = 512 * ts + 128 * k
                            xt_ = p1a.tile([128, C], f32r, tag=f"xnat{k}")
                            nc.sync.dma_start(out=xt_, in_=x_in[t0:t0 + 128, :])
                            xn.append(xt_)
                    xtc = []
                    for c in range(8):
                        tp = psP.tile([128, 512], f32, tag="p1")
                        for k in range(4):
                            nc.tensor.transpose(
                                tp[:, 128 * k:128 * (k + 1)].bitcast(f32r),
                                xn[k][:, 128 * c:128 * (c + 1)],
                                identr,
                            )
                        xc = p1b.tile([128, 512], f32r, tag=f"xtc{c}")
                        nc.scalar.activation(xc, tp, CPY)
                        xtc.append(xc)
                    for m in range(4):
                        qk = psP.tile([128, 512], f32, tag="p1")
                        for c in range(8):
                            nc.tensor.matmul(
                                qk,
                                wqk_sb[c][:, 128 * m:128 * (m + 1)],
                                xtc[c],
                                start=(c == 0),
                                stop=(c == 7),
                            )
                        dest = QP if m < 2 else KP
                        h0 = 2 * (m % 2)
                        tsl = slice(512 * ts, 512 * (ts + 1))
                        nc.vector.tensor_scalar_add(
                            dest[h0][0:64, tsl], qk[0:64, :], bqk_sb[0:64, m:m + 1]
                        )
                        nc.vector.tensor_scalar_add(
                            dest[h0 + 1][64:128, tsl], qk[64:128, :], bqk_sb[64:128, m:m + 1]
                        )
                    for k in range(4):
                        jj = 4 * ts + k
                        vp = psP.tile([128, 512], f32, tag="p1")
                        for c in range(8):
                            nc.tensor.matmul(
                                vp[:, 0:256],
                                xtc[c][:, 128 * k:128 * (k + 1)],
                                wv_sb[c],
                                start=(c == 0),
                                stop=False,
                            )
                        nc.tensor.matmul(vp[:, 0:256], ones_t, bv_sb, start=False, stop=True)
                        nc.vector.tensor_copy(
                            VP[jj][:, :, 0:64],
                            vp[:, 0:256].rearrange("p (h d) -> p h d", h=HL),
                        )

                def normalize(h, i, yt):
                    """Evacuate Y psum, divide by denominator row, store to PAIR."""
                    ysb = p2.tile([65, 512], f32, tag="ysb")
                    nc.vector.tensor_copy(ysb, yt)  # frees the psum bank fast
                    den = p2.tile([1, 512], f32, tag="den")
                    nc.sync.dma_start(out=den, in_=ysb[64:65, :])
                    rr = p2.tile([1, 512], f32, tag="rr")
                    nc.vector.reciprocal_approx_fast(out=rr, in_=den)
                    rbc = p2.tile([64, 512], f32, tag="rbc")
                    nc.gpsimd.partition_broadcast(out_ap=rbc, in_ap=rr)
                    tsl = slice(512 * i, 512 * (i + 1))
                    if h % 2 == 0:
                        nc.vector.tensor_mul(PAIR[h // 2][0:64, tsl], ysb[0:64, :], rbc)
                    else:
                        stg = p2.tile([64, 512], f32r, tag="stg")
                        nc.vector.tensor_mul(stg, ysb[0:64, :], rbc)
                        nc.sync.dma_start(out=PAIR[h // 2][64:128, tsl], in_=stg)

                def project(i):
                    """Output projection for t-blocks of t-tile i (all heads done)."""
                    for tb in range(4 * i, 4 * i + 4):
                        fp = psF[0].tile([128, 1024], f32, tag="fp")
                        tsl = slice(128 * tb, 128 * (tb + 1))
                        for n in range(2):
                            nsl = slice(512 * n, 512 * (n + 1))
                            for p in range(2):
                                nc.tensor.matmul(
                                    fp[:, nsl],
                                    PAIR[p][:, tsl],
                                    wp_sb[p][:, nsl],
                                    start=(p == 0),
                                    stop=(p == 1),
                                )
                        ob = p3.tile([128, 1024], f32, tag="ob")
                        nc.vector.tensor_copy(ob, fp)
                        nc.sync.dma_start(out=out_dram[tsl, :], in_=ob)

                # Slot h holds global heads {h*4+g : g}; the flattest slope in
                # slot h is 2^(-2(h+1)), so keys further than DELTA[h] behind
                # the query contribute < e^-32 of the softmax mass -> skip.
                DELTA = [12 * 4 ** (h + 1) for h in range(HL)]

                def emit_att(th, hs, proj_after=()):
                    tbase = 1024 * th
                    ilo_half, ihi_half = 2 * th, 2 * th + 2
                    for h in hs:
                        rows = slice(0, 67) if h % 2 == 0 else slice(0, 128)
                        Y = {}
                        started = set()

                        def front(j, i, diag):
                            """Scores + exp + causal mask for item (j, i)."""
                            n0 = 128 * (j % 4) if diag else 0
                            # full-width matmul: a PSUM start=True write that
                            # begins at a nonzero offset does not clear the
                            # region under pool-buffer reuse (stale data would
                            # be accumulated), so always write [0:512]
                            S = ps2a.tile([128, 512], f32, tag="sc")
                            nc.tensor.matmul(
                                S,
                                KP[h][rows, 128 * j:128 * (j + 1)],
                                QP[h][rows, 512 * i:512 * (i + 1)],
                                start=True,
                                stop=True,
                            )
                            PT = p2pt.tile([128, 512], f32r, tag="pt")
                            nc.scalar.activation(PT[:, n0:512], S[:, n0:512], EXP)
                            if diag:
                                nc.gpsimd.affine_select(
                                    out=PT[:, n0:n0 + 128],
                                    in_=PT[:, n0:n0 + 128],
                                    compare_op=mybir.AluOpType.is_ge,
                                    fill=0.0,
                                    base=0,
                                    pattern=[[1, 128]],
                                    channel_multiplier=-1,
                                )
                            return PT

                        def back(j, i, diag, PT):
                            """P@V accumulation (+normalize/project hooks)."""
                            ya = 128 * (j % 4) if diag else 0
                            if i not in Y:
                                Y[i] = ps2b.tile(
                                    [65, 512], f32,
                                    tag=f"yb{i % 2}", name=f"Y{h}_{i}",
                                )
                            nc.tensor.matmul(
                                Y[i][:, ya:512],
                                VP[j][:, h, :],
                                PT[:, ya:512],
                                start=(i not in started),
                                stop=(j == 4 * i + 3),
                            )
                            started.add(i)
                            if j == 4 * i + 3:
                                normalize(h, i, Y.pop(i))
                                if h == hs[-1] and i in proj_after:
                                    project(i)

                        # two-deep software pipeline over (j, i) items: the
                        # next blocks' scores run on PE while the Act engine
                        # exps this block, so P@V never waits out the full
                        # exp latency
                        items = []
                        for j in range(8 * th + 8):
                            i0 = j // 4
                            for i in range(max(i0, ilo_half), ihi_half):
                                if 128 * j + 127 >= 512 * i - DELTA[h]:
                                    items.append((j, i, i == i0))
                        PIPE = 2
                        pts = {}
                        for idx, it in enumerate(items):
                            pts[idx] = front(*it)
                            if idx >= PIPE:
                                back(*items[idx - PIPE], pts.pop(idx - PIPE))
                        for idx in range(max(0, len(items) - PIPE), len(items)):
                            back(*items[idx], pts.pop(idx))

                # --- interleaved emission ---
                emit_ts(0)
                emit_ts(1)
                emit_att(0, [0, 1])
                emit_ts(2)
                emit_att(0, [2, 3])
                emit_ts(3)
                psP.release()
                p1b.release()
                p1a.release()
                psF[0] = tc.alloc_tile_pool(name="psF", bufs=1, space="PSUM")
                project(0)
                project(1)
                emit_att(1, [0, 1, 2, 3], proj_after=(2, 3))
                psF[0].release()
                ps2b.release()
                ps2a.release()
                p3.release()
                p2pt.release()
                p2.release()




    nc.finalize()
    return nc


def _get_program():
    if "nc" not in _prog_cache:
        _prog_cache["nc"] = _build_program()
    return _prog_cache["nc"]


def _prep_core_inputs(core, x, w_attn, b_attn, w_proj):
    b, g = core // 4, core % 4
    # slot i holds global head g + 4*i (slopes grouped by magnitude per slot)
    heads = [g + 4 * i for i in range(HL)]
    qc = [slice((0 * H + h) * D, (0 * H + h) * D + D) for h in heads]
    kc = [slice((1 * H + h) * D, (1 * H + h) * D + D) for h in heads]
    vc = [slice((2 * H + h) * D, (2 * H + h) * D + D) for h in heads]

    wq = np.concatenate([w_attn[:, s] for s in qc], 1) * 0.125
    wk = np.concatenate([w_attn[:, s] for s in kc], 1)
    wqk = np.concatenate([wq, wk], 1).astype(np.float32)          # [C, 512]
    wv = np.concatenate([w_attn[:, s] for s in vc], 1).astype(np.float32)
    bq = np.concatenate([b_attn[s] for s in qc]) * 0.125
    bk = np.concatenate([b_attn[s] for s in kc])
    bqk = np.concatenate([bq, bk]).astype(np.float32).reshape(4, 128).T.copy()
    bv = np.concatenate([b_attn[s] for s in vc]).astype(np.float32)[None, :]
    wp = np.concatenate([w_proj[s, :] for s in qc], 0).astype(np.float32)  # [256, C]

    slopes = 2.0 ** (-(8.0 / H) * (np.array(heads, np.float64) + 1.0))
    pos = np.arange(T, dtype=np.float64)
    kaug = slopes[:, None] * pos[None, :]                          # [HL, T]
    khi = _round_keep9(kaug)
    klo = (kaug - khi.astype(np.float64)).astype(np.float32)
    qaug = (-(kaug + COFF)).astype(np.float32)

    augq = np.zeros((HL, 32, T), np.float32)
    augq[:, 29, :] = 1.0
    augq[:, 30, :] = 1.0
    augq[:, 31, :] = qaug
    augk = np.zeros((HL, 32, T), np.float32)
    augk[:, 29, :] = khi
    augk[:, 30, :] = klo
    augk[:, 31, :] = 1.0

    return {
        "x": np.ascontiguousarray(x[b], np.float32),
        "wqk": wqk,
        "wv": wv,
        "wp": np.ascontiguousarray(wp),
        "bqk": bqk,
        "bv": bv,
        "augq": augq,
        "augk": augk,
    }


def kernel(x, w_attn, b_attn, w_proj, b_proj, _run_kwargs=None):
    from concourse.bass_utils import run_bass_kernel_spmd

    x = np.asarray(x, np.float32)
    w_attn = np.asarray(w_attn, np.float32)
    b_attn = np.asarray(b_attn, np.float32)
    w_proj = np.asarray(w_proj, np.float32)
    b_proj = np.asarray(b_proj, np.float32)

    nc = _get_program()
    in_maps = [_prep_core_inputs(c, x, w_attn, b_attn, w_proj) for c in range(NCORES)]
    res = run_bass_kernel_spmd(
        nc, in_maps, core_ids=list(range(NCORES)), **(_run_kwargs or {})
    )
    _prog_cache["last_result"] = res

    out = np.zeros((B, T, C), np.float32)
    for c in range(NCORES):
        out[c // 4] += res.results[c]["out"]
    out += b_proj[None, None, :]
    return out



# revision 49
# speedup vs baseline: 1.0368x; 1.0152x over previous
"""Causal self-attention with ALiBi, sharded over 8 TRN2 NeuronCores.

Sharding: core c -> batch b = c//4, head group g = c%4 (4 heads each).
Each core computes QKV projection for its heads, causal attention, and the
partial output projection (w_proj rows of its heads). Host sums the 4
partials per batch and adds b_proj.

Kernel math tricks (all folded into matmuls so softmax is one exp pass):
  - scores are computed TRANSPOSED (s on partitions, t free) so exp(S^T)=P^T
    lands exactly in the lhsT layout the P@V matmul needs.
  - ALiBi bias slope*s, the stability offset -(slope*t + c), and the /sqrt(D)
    scale are folded into 3 extra contraction rows of the QK^T matmul
    (q' = [q/8, 1, 1, -(slope*t+c)], k' = [k, hi, lo, 1] with hi+lo an exact
    split of slope*s to survive f32r rounding).
  - V is augmented with a ones column so the softmax denominator appears as
    row 64 of the (unnormalized) y^T accumulator.
  - normalization commutes with the head-dim contraction, applied via
    reciprocal + partition broadcast before the output projection.
All matmuls run in f32r (single-pass fp32, ~1e-4 rel err).
"""

import numpy as np

B, T, C, H = 2, 2048, 1024, 16
D = C // H          # 64
HL = 4              # heads per core
NCORES = 8
COFF = 8.0          # softmax stability offset

_prog_cache = {}


def _round_keep9(x):
    """RNE to 9 explicit mantissa bits (exactly representable in f32r)."""
    b = np.asarray(x, np.float32).view(np.uint32)
    half = np.uint32(1 << 13)
    mask = np.uint32(0xFFFFFFFF) << 14
    return ((b + half) & mask).view(np.float32)


def _build_program():
    import concourse.bass as bass  # noqa: F401
    import concourse.mybir as mybir
    import concourse.tile as tile
    from concourse import bacc

    f32 = mybir.dt.float32
    f32r = mybir.dt.float32r
    EXP = mybir.ActivationFunctionType.Exp
    CPY = mybir.ActivationFunctionType.Copy

    nc = bacc.Bacc("TRN2", target_bir_lowering=False, num_devices=NCORES)

    x_in = nc.declare_dram_parameter("x", [T, C], f32r, isOutput=False)
    wqk_in = nc.declare_dram_parameter("wqk", [C, 512], f32r, isOutput=False)
    wv_in = nc.declare_dram_parameter("wv", [C, 256], f32r, isOutput=False)
    wp_in = nc.declare_dram_parameter("wp", [256, C], f32r, isOutput=False)
    bqk_in = nc.declare_dram_parameter("bqk", [128, 4], f32, isOutput=False)
    bv_in = nc.declare_dram_parameter("bv", [1, 256], f32r, isOutput=False)
    # aug rows per head: [.., 29:32, :] = the 3 aug rows ([1,1,qaug] q-side,
    # [khi,klo,1] k-side); rows 0:29 are zeros (odd-head padding).
    augq_in = nc.declare_dram_parameter("augq", [HL, 32, T], f32r, isOutput=False)
    augk_in = nc.declare_dram_parameter("augk", [HL, 32, T], f32r, isOutput=False)
    out_dram = nc.declare_dram_parameter("out", [T, C], f32, isOutput=True)

    with tile.TileContext(nc) as tc:
        with (
            tc.tile_pool(name="persist", bufs=1) as pp,
            tc.tile_pool(name="consts", bufs=1) as cp,
        ):
            # ---- constants / weights ----
            from concourse.masks import make_identity

            ident = cp.tile([128, 128], f32)
            make_identity(nc, ident)
            identr = cp.tile([128, 128], f32r)
            nc.vector.tensor_copy(identr, ident)

            # prefetch the first t-super of x before anything else so the
            # transposes (first PE work) start as early as possible
            p2 = tc.alloc_tile_pool(name="ph2", bufs=2)
            p2pt = tc.alloc_tile_pool(name="ph2pt", bufs=3)
            p3 = tc.alloc_tile_pool(name="ph3", bufs=2)
            ps2a = tc.alloc_tile_pool(name="ps2a", bufs=3, space="PSUM")
            ps2b = tc.alloc_tile_pool(name="ps2b", bufs=1, space="PSUM")
            p1a = tc.alloc_tile_pool(name="ph1a", bufs=1)
            p1b = tc.alloc_tile_pool(name="ph1b", bufs=1)
            psP = tc.alloc_tile_pool(name="psP", bufs=2, space="PSUM")
            xn0 = []
            for k in range(4):
                xt_ = p1a.tile([128, C], f32r, tag=f"xnat{k}")
                nc.sync.dma_start(out=xt_, in_=x_in[128 * k:128 * (k + 1), :])
                xn0.append(xt_)

            wqk_sb = [cp.tile([128, 512], f32r, name=f"wqk{c}", tag=f"wqk{c}") for c in range(8)]
            for c in range(8):
                nc.sync.dma_start(out=wqk_sb[c], in_=wqk_in[128 * c:128 * (c + 1), :])
            wv_sb = [cp.tile([128, 256], f32r, name=f"wv{c}", tag=f"wv{c}") for c in range(8)]
            for c in range(8):
                nc.sync.dma_start(out=wv_sb[c], in_=wv_in[128 * c:128 * (c + 1), :])
            bqk_sb = cp.tile([128, 4], f32)
            nc.sync.dma_start(out=bqk_sb, in_=bqk_in[:, :])
            bv_sb = cp.tile([1, 256], f32r)
            nc.sync.dma_start(out=bv_sb, in_=bv_in[:, :])
            ones_t = cp.tile([1, 128], f32r)
            nc.vector.memset(ones_t.bitcast(f32), 1.0)

            # ---- persistent attention operands ----
            # Q'/K' per head: [128, T]. Even local head: rows 0-63 head data,
            # rows 64-66 augs. Odd local head: rows 61-63 augs, 64-127 data.
            QP = [pp.tile([128, T], f32r, name=f"QP{h}", tag=f"QP{h}") for h in range(HL)]
            KP = [pp.tile([128, T], f32r, name=f"KP{h}", tag=f"KP{h}") for h in range(HL)]
            # V' per s-block: [128, HL, 65] (cols 0-63 = v, col 64 = ones)
            VP = [pp.tile([128, HL, 65], f32r, name=f"VP{j}", tag=f"VP{j}") for j in range(16)]
            # normalized y^T stacked per head pair: [128, T]
            PAIR = [pp.tile([128, T], f32r, name=f"PAIR{p}", tag=f"PAIR{p}") for p in range(2)]

            for h in range(HL):
                if h % 2 == 0:
                    # rows 64-66 = augs; contraction slice [0:67]
                    nc.sync.dma_start(out=QP[h][64:67, :], in_=augq_in[h, 29:32, :])
                    nc.sync.dma_start(out=KP[h][64:67, :], in_=augk_in[h, 29:32, :])
                else:
                    # contraction slice [0:128]: rows 0-60 zero, 61-63 augs,
                    # 64-127 data (zero rows cost nothing: PE time ~ N only)
                    nc.vector.memset(QP[h][0:32, :].bitcast(f32), 0.0)
                    nc.vector.memset(KP[h][0:32, :].bitcast(f32), 0.0)
                    nc.sync.dma_start(out=QP[h][32:64, :], in_=augq_in[h, :, :])
                    nc.sync.dma_start(out=KP[h][32:64, :], in_=augk_in[h, :, :])
            for j in range(16):
                nc.vector.memset(VP[j][:, :, 64:65].bitcast(f32), 1.0)

            # ===== interleaved pipeline: projections feed attention =====
            # PSUM budget (8 banks): p1 shared proj staging (2) + scores (4)
            # + y accumulators (2); after phase-1 release, fp takes p1's banks.
            psF = [None]

            if True:
                wp_sb = [p3.tile([128, C], f32r, name=f"wp{p}", tag=f"wp{p}") for p in range(2)]
                for p in range(2):
                    nc.sync.dma_start(out=wp_sb[p], in_=wp_in[128 * p:128 * (p + 1), :])

                def emit_ts(ts):
                    if ts == 0:
                        xn = xn0
                    else:
                        xn = []
                        for k in range(4):
                            t0 = 512 * ts + 128 * k
                            xt_ = p1a.tile([128, C], f32r, tag=f"xnat{k}")
                            nc.sync.dma_start(out=xt_, in_=x_in[t0:t0 + 128, :])
                            xn.append(xt_)
                    xtc = []
                    for c in range(8):
                        tp = psP.tile([128, 512], f32, tag="p1")
                        for k in range(4):
                            nc.tensor.transpose(
                                tp[:, 128 * k:128 * (k + 1)].bitcast(f32r),
                                xn[k][:, 128 * c:128 * (c + 1)],
                                identr,
                            )
                        xc = p1b.tile([128, 512], f32r, tag=f"xtc{c}")
                        if c % 2 == 0:
                            nc.scalar.activation(xc, tp, CPY)
                        else:
                            nc.vector.tensor_copy(xc, tp)
                        xtc.append(xc)
                    for m in range(4):
                        qk = psP.tile([128, 512], f32, tag="p1")
                        for c in range(8):
                            nc.tensor.matmul(
                                qk,
                                wqk_sb[c][:, 128 * m:128 * (m + 1)],
                                xtc[c],
                                start=(c == 0),
                                stop=(c == 7),
                            )
                        dest = QP if m < 2 else KP
                        h0 = 2 * (m % 2)
                        tsl = slice(512 * ts, 512 * (ts + 1))
                        nc.vector.tensor_scalar_add(
                            dest[h0][0:64, tsl], qk[0:64, :], bqk_sb[0:64, m:m + 1]
                        )
                        nc.vector.tensor_scalar_add(
                            dest[h0 + 1][64:128, tsl], qk[64:128, :], bqk_sb[64:128, m:m + 1]
                        )
                    for k in range(4):
                        jj = 4 * ts + k
                        vp = psP.tile([128, 512], f32, tag="p1")
                        for c in range(8):
                            nc.tensor.matmul(
                                vp[:, 0:256],
                                xtc[c][:, 128 * k:128 * (k + 1)],
                                wv_sb[c],
                                start=(c == 0),
                                stop=False,
                            )
                        nc.tensor.matmul(vp[:, 0:256], ones_t, bv_sb, start=False, stop=True)
                        nc.vector.tensor_copy(
                            VP[jj][:, :, 0:64],
                            vp[:, 0:256].rearrange("p (h d) -> p h d", h=HL),
                        )

                def normalize(h, i, yt):
                    """Evacuate Y psum, divide by denominator row, store to PAIR."""
                    ysb = p2.tile([65, 512], f32, tag="ysb")
                    nc.vector.tensor_copy(ysb, yt)  # frees the psum bank fast
                    den = p2.tile([1, 512], f32, tag="den")
                    nc.sync.dma_start(out=den, in_=ysb[64:65, :])
                    rr = p2.tile([1, 512], f32, tag="rr")
                    nc.vector.reciprocal_approx_fast(out=rr, in_=den)
                    rbc = p2.tile([64, 512], f32, tag="rbc")
                    nc.gpsimd.partition_broadcast(out_ap=rbc, in_ap=rr)
                    tsl = slice(512 * i, 512 * (i + 1))
                    if h % 2 == 0:
                        nc.vector.tensor_mul(PAIR[h // 2][0:64, tsl], ysb[0:64, :], rbc)
                    else:
                        stg = p2.tile([64, 512], f32r, tag="stg")
                        nc.vector.tensor_mul(stg, ysb[0:64, :], rbc)
                        nc.sync.dma_start(out=PAIR[h // 2][64:128, tsl], in_=stg)

                def project(i):
                    """Output projection for t-blocks of t-tile i (all heads done)."""
                    for tb in range(4 * i, 4 * i + 4):
                        fp = psF[0].tile([128, 1024], f32, tag="fp")
                        tsl = slice(128 * tb, 128 * (tb + 1))
                        for n in range(2):
                            nsl = slice(512 * n, 512 * (n + 1))
                            for p in range(2):
                                nc.tensor.matmul(
                                    fp[:, nsl],
                                    PAIR[p][:, tsl],
                                    wp_sb[p][:, nsl],
                                    start=(p == 0),
                                    stop=(p == 1),
                                )
                        ob = p3.tile([128, 1024], f32, tag="ob")
                        nc.vector.tensor_copy(ob, fp)
                        nc.sync.dma_start(out=out_dram[tsl, :], in_=ob)

                # Slot h holds global heads {h*4+g : g}; the flattest slope in
                # slot h is 2^(-2(h+1)), so keys further than DELTA[h] behind
                # the query contribute < e^-32 of the softmax mass -> skip.
                DELTA = [12 * 4 ** (h + 1) for h in range(HL)]

                def emit_att(th, hs, proj_after=()):
                    tbase = 1024 * th
                    ilo_half, ihi_half = 2 * th, 2 * th + 2
                    for h in hs:
                        rows = slice(0, 67) if h % 2 == 0 else slice(0, 128)
                        Y = {}
                        started = set()

                        def front(j, i, diag):
                            """Scores + exp + causal mask for item (j, i)."""
                            n0 = 128 * (j % 4) if diag else 0
                            # full-width matmul: a PSUM start=True write that
                            # begins at a nonzero offset does not clear the
                            # region under pool-buffer reuse (stale data would
                            # be accumulated), so always write [0:512]
                            S = ps2a.tile([128, 512], f32, tag="sc")
                            nc.tensor.matmul(
                                S,
                                KP[h][rows, 128 * j:128 * (j + 1)],
                                QP[h][rows, 512 * i:512 * (i + 1)],
                                start=True,
                                stop=True,
                            )
                            PT = p2pt.tile([128, 512], f32r, tag="pt")
                            nc.scalar.activation(PT[:, n0:512], S[:, n0:512], EXP)
                            if diag:
                                nc.gpsimd.affine_select(
                                    out=PT[:, n0:n0 + 128],
                                    in_=PT[:, n0:n0 + 128],
                                    compare_op=mybir.AluOpType.is_ge,
                                    fill=0.0,
                                    base=0,
                                    pattern=[[1, 128]],
                                    channel_multiplier=-1,
                                )
                            return PT

                        def back(j, i, diag, PT):
                            """P@V accumulation (+normalize/project hooks)."""
                            ya = 128 * (j % 4) if diag else 0
                            if i not in Y:
                                Y[i] = ps2b.tile(
                                    [65, 512], f32,
                                    tag=f"yb{i % 2}", name=f"Y{h}_{i}",
                                )
                            nc.tensor.matmul(
                                Y[i][:, ya:512],
                                VP[j][:, h, :],
                                PT[:, ya:512],
                                start=(i not in started),
                                stop=(j == 4 * i + 3),
                            )
                            started.add(i)
                            if j == 4 * i + 3:
                                normalize(h, i, Y.pop(i))
                                if h == hs[-1] and i in proj_after:
                                    project(i)

                        # two-deep software pipeline over (j, i) items: the
                        # next blocks' scores run on PE while the Act engine
                        # exps this block, so P@V never waits out the full
                        # exp latency
                        items = []
                        for j in range(8 * th + 8):
                            i0 = j // 4
                            for i in range(max(i0, ilo_half), ihi_half):
                                if 128 * j + 127 >= 512 * i - DELTA[h]:
                                    items.append((j, i, i == i0))
                        PIPE = 2
                        pts = {}
                        for idx, it in enumerate(items):
                            pts[idx] = front(*it)
                            if idx >= PIPE:
                                back(*items[idx - PIPE], pts.pop(idx - PIPE))
                        for idx in range(max(0, len(items) - PIPE), len(items)):
                            back(*items[idx], pts.pop(idx))

                # --- interleaved emission ---
                emit_ts(0)
                emit_ts(1)
                emit_att(0, [0, 1])
                emit_ts(2)
                emit_att(0, [2, 3])
                emit_ts(3)
                psP.release()
                p1b.release()
                p1a.release()
                psF[0] = tc.alloc_tile_pool(name="psF", bufs=1, space="PSUM")
                project(0)
                project(1)
                emit_att(1, [0, 1, 2, 3], proj_after=(2, 3))
                psF[0].release()
                ps2b.release()
                ps2a.release()
                p3.release()
                p2pt.release()
                p2.release()




    nc.finalize()
    return nc


def _get_program():
    if "nc" not in _prog_cache:
        _prog_cache["nc"] = _build_program()
    return _prog_cache["nc"]


def _prep_core_inputs(core, x, w_attn, b_attn, w_proj):
    b, g = core // 4, core % 4
    # slot i holds global head g + 4*i (slopes grouped by magnitude per slot)
    heads = [g + 4 * i for i in range(HL)]
    qc = [slice((0 * H + h) * D, (0 * H + h) * D + D) for h in heads]
    kc = [slice((1 * H + h) * D, (1 * H + h) * D + D) for h in heads]
    vc = [slice((2 * H + h) * D, (2 * H + h) * D + D) for h in heads]

    wq = np.concatenate([w_attn[:, s] for s in qc], 1) * 0.125
    wk = np.concatenate([w_attn[:, s] for s in kc], 1)
    wqk = np.concatenate([wq, wk], 1).astype(np.float32)          # [C, 512]
    wv = np.concatenate([w_attn[:, s] for s in vc], 1).astype(np.float32)
    bq = np.concatenate([b_attn[s] for s in qc]) * 0.125
    bk = np.concatenate([b_attn[s] for s in kc])
    bqk = np.concatenate([bq, bk]).astype(np.float32).reshape(4, 128).T.copy()
    bv = np.concatenate([b_attn[s] for s in vc]).astype(np.float32)[None, :]
    wp = np.concatenate([w_proj[s, :] for s in qc], 0).astype(np.float32)  # [256, C]

    slopes = 2.0 ** (-(8.0 / H) * (np.array(heads, np.float64) + 1.0))
    pos = np.arange(T, dtype=np.float64)
    kaug = slopes[:, None] * pos[None, :]                          # [HL, T]
    khi = _round_keep9(kaug)
    klo = (kaug - khi.astype(np.float64)).astype(np.float32)
    qaug = (-(kaug + COFF)).astype(np.float32)

    augq = np.zeros((HL, 32, T), np.float32)
    augq[:, 29, :] = 1.0
    augq[:, 30, :] = 1.0
    augq[:, 31, :] = qaug
    augk = np.zeros((HL, 32, T), np.float32)
    augk[:, 29, :] = khi
    augk[:, 30, :] = klo
    augk[:, 31, :] = 1.0

    return {
        "x": np.ascontiguousarray(x[b], np.float32),
        "wqk": wqk,
        "wv": wv,
        "wp": np.ascontiguousarray(wp),
        "bqk": bqk,
        "bv": bv,
        "augq": augq,
        "augk": augk,
    }


def kernel(x, w_attn, b_attn, w_proj, b_proj, _run_kwargs=None):
    from concourse.bass_utils import run_bass_kernel_spmd

    x = np.asarray(x, np.float32)
    w_attn = np.asarray(w_attn, np.float32)
    b_attn = np.asarray(b_attn, np.float32)
    w_proj = np.asarray(w_proj, np.float32)
    b_proj = np.asarray(b_proj, np.float32)

    nc = _get_program()
    in_maps = [_prep_core_inputs(c, x, w_attn, b_attn, w_proj) for c in range(NCORES)]
    res = run_bass_kernel_spmd(
        nc, in_maps, core_ids=list(range(NCORES)), **(_run_kwargs or {})
    )
    _prog_cache["last_result"] = res

    out = np.zeros((B, T, C), np.float32)
    for c in range(NCORES):
        out[c // 4] += res.results[c]["out"]
    out += b_proj[None, None, :]
    return out

